# revision 14
# baseline (speedup 1.0000x reference)
"""Trainium2 Bass kernel for nn_Cifar10ConvBNN (binarized CNN, batch-stat BN).

Strategy (8-core data parallel, 32 images/core):
 - All post-sign activations are +-1 -> conv2-6 and fc matmuls run in fp8/bf16
   with exact f32 PSUM accumulation (integer sums, < 2^24).
 - BN counts are powers of two -> channel means are exact dyadics; with
   gamma=1, beta=0 the sign() after BN is sign(x - mean), which this kernel
   reproduces BIT-EXACTLY vs the f32 jax reference for layers 2..fc2.
 - Layer 1 (continuous input) uses a hi/lo fixed-point split of x
   (hi on a 2^-14 grid -> exact PE sums; lo tiny) and folds -mean into the
   matmul as an extra K-row, making the sign decision accurate to ~1e-9 --
   far below the minimum |x - m| gap (2e-8) of this problem instance.
 - Cross-core batch stats via one small AllReduce per layer (9 total).
 - maxpool commutes with the monotone BN affine -> pool pre-BN values, then
   one fused Sign(x + (-m)) activation writes the next layer's padded input.
"""

import numpy as np

import concourse.bass as bass
import concourse.mybir as mybir
import concourse.tile as tile
from concourse import bacc
from concourse.bass_utils import run_bass_kernel_spmd
from concourse.masks import make_identity

F32 = mybir.dt.float32
F16 = mybir.dt.float16
BF16 = mybir.dt.bfloat16
FP8 = mybir.dt.float8e4
AF = mybir.ActivationFunctionType
AX = mybir.AxisListType

N_CORES = 8
B = 32  # images per core
NB_FULL = 256

# conv layer configs (layers 2..6): cin_tiles, cout_tiles, H(=W), pool?
CONV_CFG = {
    2: dict(ci=1, co=1, H=32, pool=True),
    3: dict(ci=2, co=2, H=16, pool=False),
    4: dict(ci=2, co=2, H=16, pool=True),
    5: dict(ci=4, co=4, H=8, pool=False),
    6: dict(ci=4, co=4, H=8, pool=True),
}
# wait: layer 3 is 128->256 (ci=1), layer 5 is 256->512 (ci=2).
CONV_CFG[3] = dict(ci=1, co=2, H=16, pool=False)
CONV_CFG[5] = dict(ci=2, co=4, H=8, pool=False)

_RUNNER = {}


def _taps():
    return [(ky, kx) for ky in range(3) for kx in range(3)]


def build():
    nc = bacc.Bacc("TRN2", target_bir_lowering=False, debug=False,
                   num_devices=N_CORES)

    # ---------------- DRAM parameters ----------------
    xh_p = nc.declare_dram_parameter("xh", [3, B, 34, 34], F32, isOutput=False)
    xl_p = nc.declare_dram_parameter("xl", [3, B, 34, 34], F32, isOutput=False)
    wc1_p = nc.declare_dram_parameter("wc1", [27, 128], F32, isOutput=False)
    w_p = {}
    for l in range(2, 7):
        c = CONV_CFG[l]
        w_p[l] = nc.declare_dram_parameter(
            f"w{l}", [c["ci"], 9, c["co"], 128, 128], FP8, isOutput=False)
    wf1_p = nc.declare_dram_parameter("wf1", [64, 128, 1024], FP8, isOutput=False)
    wf2_p = nc.declare_dram_parameter("wf2", [8, 128, 1024], BF16, isOutput=False)
    wf3_p = nc.declare_dram_parameter("wf3", [8, 128, 16], BF16, isOutput=False)
    gb3_p = nc.declare_dram_parameter("gb3", [16, 2], F32, isOutput=False)
    y_p = nc.declare_dram_parameter("y", [B, 10], F32, isOutput=True)

    with tile.TileContext(nc) as tc:
        dram = tc.alloc_tile_pool(name="dram", bufs=1, space="DRAM")
        psum = tc.alloc_tile_pool(name="psum", bufs=6, space="PSUM")
        pconst = tc.alloc_tile_pool(name="pconst", bufs=1)

        def ps_tile(name):
            return psum.tile([128, 512], F32, name=name, tag="ps")

        # identities for PE transposes
        ident128 = pconst.tile([128, 128], F32)
        make_identity(nc, ident128[:])
        ident32 = pconst.tile([32, 32], F32)
        make_identity(nc, ident32[:])

        # AR bounce buffers
        arin = {}
        arout = {}
        for key, shape in [("l1", [2, 128]), ("l2", [1, 128]), ("l3", [2, 128]),
                           ("l4", [2, 128]), ("l5", [4, 128]), ("l6", [4, 128]),
                           ("f1", [8, 128]), ("f2", [8, 128]), ("f3", [2, 16])]:
            arin[key] = dram.tile(shape, F32, name=f"arin_{key}")
            arout[key] = dram.tile(shape, F32, name=f"arout_{key}",
                                   addr_space="Shared")

        def allreduce(key):
            nc.gpsimd.collective_compute(
                "AllReduce", mybir.AluOpType.add,
                ins=[arin[key][:]], outs=[arout[key][:]],
                replica_groups=[list(range(N_CORES))],
            )

        # ================= LAYER 1 =================
        poolL1 = tc.alloc_tile_pool(name="poolL1", bufs=1)
        poolA1 = tc.alloc_tile_pool(name="poolA1", bufs=1, side="right")

        # persistent L1 sources [(c,b) partitions, padded pixels]
        xh_s = poolL1.tile([96, 34 * 34], F32)
        nc.sync.dma_start(xh_s[:], xh_p[:].rearrange("c b y x -> (c b) (y x)"))
        xl_s = poolL1.tile([96, 34 * 34], F32)
        nc.sync.dma_start(xl_s[:], xl_p[:].rearrange("c b y x -> (c b) (y x)"))

        wc1_sb = poolL1.tile([32, 128], F32)
        nc.sync.dma_start(wc1_sb[0:27, :], wc1_p[:])

        # block-selector for per-(c) partition sums: blk[p, c] = (p//32 == c)
        blk = poolL1.tile([96, 3], F32)
        nc.vector.memset(blk[:], 0.0)
        for c in range(3):
            nc.vector.memset(blk[c * 32:(c + 1) * 32, c:c + 1], 1.0)

        # ---- pass 1: per-(c,tap) column sums -> Sigma conv_hi/lo per cout ----
        xh_v = xh_s[:].rearrange("p (y x) -> p y x", y=34, x=34)
        xl_v = xl_s[:].rearrange("p (y x) -> p y x", y=34, x=34)
        wsum_h = poolL1.tile([96, 9], F32)
        wsum_l = poolL1.tile([96, 9], F32)
        for t, (ky, kx) in enumerate(_taps()):
            nc.vector.reduce_sum(wsum_h[:, t:t + 1],
                                 xh_v[:, ky:ky + 32, kx:kx + 32], axis=AX.XY)
            nc.vector.reduce_sum(wsum_l[:, t:t + 1],
                                 xl_v[:, ky:ky + 32, kx:kx + 32], axis=AX.XY)
        # per-(c,t) sums via tiny matmuls: out[c, t] = sum_b wsum[(c,b), t]
        cs_ps = psum.tile([128, 512], F32, name="cs_ps", tag="ps")
        for t in range(9):
            nc.tensor.matmul(cs_ps[0:3, t:t + 1], blk[:], wsum_h[:, t:t + 1],
                             start=True, stop=True)
            nc.tensor.matmul(cs_ps[0:3, 9 + t:10 + t], blk[:], wsum_l[:, t:t + 1],
                             start=True, stop=True)
        cs_sb = poolL1.tile([3, 18], F32)
        nc.scalar.copy(cs_sb[:], cs_ps[0:3, 0:18])
        colsum = poolL1.tile([32, 2], F32)  # [(c,t) 27 rows, {hi,lo}]
        nc.sync.dma_start(colsum[0:27, 0:1], cs_sb[:, 0:9])
        nc.sync.dma_start(colsum[0:27, 1:2], cs_sb[:, 9:18])
        sum_ps = psum.tile([128, 512], F32, name="sum_ps", tag="ps")
        nc.tensor.matmul(sum_ps[:, 0:1], wc1_sb[0:27, :], colsum[0:27, 0:1],
                         start=True, stop=True)
        nc.tensor.matmul(sum_ps[:, 1:2], wc1_sb[0:27, :], colsum[0:27, 1:2],
                         start=True, stop=True)
        sums_sb = poolL1.tile([128, 2], F32)
        nc.scalar.copy(sums_sb[:], sum_ps[:, 0:2])
        nc.sync.dma_start(arin["l1"][0, :], sums_sb[:, 0:1])
        nc.sync.dma_start(arin["l1"][1, :], sums_sb[:, 1:2])
        allreduce("l1")

        # -mean rows (exact: sums * -2^-18)
        mn1 = poolL1.tile([128, 2], F32)
        nc.sync.dma_start(mn1[:, 0:1], arout["l1"][0, :])
        nc.sync.dma_start(mn1[:, 1:2], arout["l1"][1, :])
        mn1n = poolL1.tile([128, 2], F32)
        nc.scalar.mul(mn1n[:], mn1[:], -1.0 / (1 << 18))
        lhsT_hi = poolL1.tile([32, 128], F32)
        nc.sync.dma_start(lhsT_hi[0:27, :], wc1_p[:])
        lhsT_lo = poolL1.tile([32, 128], F32)
        nc.sync.dma_start(lhsT_lo[0:27, :], wc1_p[:])
        tp_ps = psum.tile([128, 512], F32, name="tp_ps", tag="ps")
        nc.tensor.transpose(tp_ps[0:1, 0:128], mn1n[:, 0:1], ident128[:])
        mrow = poolL1.tile([1, 2, 128], F32)
        nc.scalar.copy(mrow[:, 0, :], tp_ps[0:1, 0:128])
        tp_ps2 = psum.tile([128, 512], F32, name="tp_ps2", tag="ps")
        nc.tensor.transpose(tp_ps2[0:1, 0:128], mn1n[:, 1:2], ident128[:])
        nc.scalar.copy(mrow[:, 1, :], tp_ps2[0:1, 0:128])
        nc.sync.dma_start(lhsT_hi[27:28, :], mrow[:, 0, :])
        nc.sync.dma_start(lhsT_lo[27:28, :], mrow[:, 1, :])

        # act1: L2 input, padded fp8 [128, B, 34, 34]
        act1 = poolA1.tile([128, B, 34, 34], FP8)
        nc.gpsimd.memset(act1[:, :, 0:1, :], 0.0)
        nc.gpsimd.memset(act1[:, :, 33:34, :], 0.0)
        nc.gpsimd.memset(act1[:, :, 1:33, 0:1], 0.0)
        nc.gpsimd.memset(act1[:, :, 1:33, 33:34], 0.0)

        # ---- pass 2: real conv via im2col chunks of 4 images ----
        ones_row = poolL1.tile([1, 4096], F32)
        nc.vector.memset(ones_row[:], 1.0)
        im2h_bufs = [poolL1.tile([32, 4, 32, 32], F32, name=f"im2h{i}")
                     for i in range(2)]
        im2l_bufs = [poolL1.tile([32, 4, 32, 32], F32, name=f"im2l{i}")
                     for i in range(2)]
        ones_v = ones_row[:].rearrange("p (b y x) -> p b y x", b=4, y=32, x=32)
        for i in range(2):
            nc.sync.dma_start(im2h_bufs[i][27:28, :, :, :], ones_v)
            nc.sync.dma_start(im2l_bufs[i][27:28, :, :, :], ones_v)
        for bc in range(8):
            b0 = bc * 4
            im2h = im2h_bufs[bc % 2]
            im2l = im2l_bufs[bc % 2]
            for t, (ky, kx) in enumerate(_taps()):
                for c in range(3):
                    r = c * 9 + t
                    src = xh_v[c * 32 + b0:c * 32 + b0 + 4,
                               ky:ky + 32, kx:kx + 32]
                    nc.sync.dma_start(im2h[r:r + 1, :, :, :], src)
                    srcl = xl_v[c * 32 + b0:c * 32 + b0 + 4,
                                ky:ky + 32, kx:kx + 32]
                    nc.sync.dma_start(im2l[r:r + 1, :, :, :], srcl)
            for bi in range(4):
                for half in range(2):
                    ys = half * 16
                    ps = ps_tile("l1ps")
                    nc.tensor.matmul(ps[:, :],
                                     lhsT_hi[0:28, :],
                                     im2h[0:28, bi, ys:ys + 16, :],
                                     start=True, stop=False)
                    nc.tensor.matmul(ps[:, :],
                                     lhsT_lo[0:28, :],
                                     im2l[0:28, bi, ys:ys + 16, :],
                                     start=False, stop=True)
                    psv = ps[:].rearrange("p (y x) -> p y x", y=16, x=32)
                    nc.scalar.activation(
                        act1[:, b0 + bi, 1 + ys:17 + ys, 1:33], psv,
                        AF.Sign, bias=0.0, scale=1.0)
        poolL1.release()

        # ================= CONV LAYERS 2..6 =================
        def conv_layer(l, acts_in, key, nbn_log2, pt, release_after_mm=None):
            """acts_in: list of padded fp8 input tiles [128, B, H+2, W+2].
            Emits matmuls+stats+AR; tiles go into pool pt."""
            cfg = CONV_CFG[l]
            CI, CO, H, do_pool = cfg["ci"], cfg["co"], cfg["H"], cfg["pool"]
            HP = H + 2
            # chunks: (b_slice, y_slice covering 512 px)
            if H == 32:
                chunks = [(bi, 1, hy * 16, 16, 32) for bi in range(B)
                          for hy in range(2)]  # (b0, nb, y0, ny, nx)
            elif H == 16:
                chunks = [(2 * j, 2, 0, 16, 16) for j in range(16)]
            else:
                chunks = [(8 * j, 8, 0, 8, 8) for j in range(4)]
            NCH = len(chunks)
            Hq, Wq = (H // 2, H // 2) if do_pool else (H, H)

            wsb = []
            for ci in range(CI):
                wt = pt.tile([128, 9 * CO, 128], FP8, name=f"w{l}_{ci}")
                nc.sync.dma_start(wt[:], w_p[l][ci].rearrange(
                    "t co kc ko -> kc (t co) ko"))
                wsb.append(wt)
            xh_tiles = [pt.tile([128, B, Hq, Wq], F16, name=f"xh{l}_{co}")
                        for co in range(CO)]
            csum = pt.tile([128, CO * NCH], F32, name=f"cs{l}")
            tot = pt.tile([128, CO], F32, name=f"tot{l}")

            for co in range(CO):
                for ich, (b0, nb, y0, ny, nx) in enumerate(chunks):
                    ps = ps_tile(f"l{l}ps")
                    nmm = 9 * CI
                    k = 0
                    for t, (ky, kx) in enumerate(_taps()):
                        for ci in range(CI):
                            rhs = acts_in[ci][:, b0:b0 + nb,
                                              y0 + ky:y0 + ky + ny,
                                              kx:kx + nx]
                            nc.tensor.matmul(ps[:, :],
                                             wsb[ci][:, t * CO + co, :],
                                             rhs, start=(k == 0),
                                             stop=(k == nmm - 1))
                            k += 1
                    nc.vector.reduce_sum(csum[:, co * NCH + ich:co * NCH + ich + 1],
                                         ps[:, :], axis=AX.X)
                    psv = ps[:].rearrange("p (b y x) -> p b y x",
                                          b=nb, y=ny, x=nx)
                    if do_pool:
                        stage = pt.tile([128, nb, ny, nx], F16,
                                        name=f"st{l}", tag=f"st{l}", bufs=4)
                        nc.scalar.copy(stage[:], psv)
                        t1 = pt.tile([128, nb, ny, nx // 2], F16,
                                     name=f"t1{l}", tag=f"t1{l}", bufs=4)
                        nc.vector.tensor_max(t1[:], stage[:, :, :, 0::2],
                                             stage[:, :, :, 1::2])
                        nc.vector.tensor_max(
                            xh_tiles[co][:, b0:b0 + nb,
                                         y0 // 2:y0 // 2 + ny // 2, :],
                            t1[:, :, 0::2, :], t1[:, :, 1::2, :])
                    else:
                        nc.scalar.copy(
                            xh_tiles[co][:, b0:b0 + nb, y0:y0 + ny, :], psv)
                # layer totals for this cout tile
                nc.vector.reduce_sum(tot[:, co:co + 1],
                                     csum[:, co * NCH:(co + 1) * NCH], axis=AX.X)
                nc.sync.dma_start(arin[key][co, :], tot[:, co:co + 1])
            if release_after_mm is not None:
                release_after_mm.release()
            allreduce(key)
            mn = pt.tile([128, CO], F32, name=f"mn{l}")
            mneg = pt.tile([128, CO], F32, name=f"mneg{l}")
            for co in range(CO):
                nc.sync.dma_start(mn[:, co:co + 1], arout[key][co, :])
            nc.scalar.mul(mneg[:], mn[:], -1.0 / (1 << nbn_log2))
            return xh_tiles, mneg, Hq

        def sign_to_padded(l, xh_tiles, mneg, Hq, pool_next):
            """Sign(xh + mneg) -> new padded fp8 act tiles in pool_next."""
            outs = []
            for co in range(len(xh_tiles)):
                a = pool_next.tile([128, B, Hq + 2, Hq + 2], FP8,
                                   name=f"act{l}_{co}")
                nc.gpsimd.memset(a[:, :, 0:1, :], 0.0)
                nc.gpsimd.memset(a[:, :, Hq + 1:Hq + 2, :], 0.0)
                nc.gpsimd.memset(a[:, :, 1:Hq + 1, 0:1], 0.0)
                nc.gpsimd.memset(a[:, :, 1:Hq + 1, Hq + 1:Hq + 2], 0.0)
                nc.scalar.activation(a[:, :, 1:Hq + 1, 1:Hq + 1],
                                     xh_tiles[co][:],
                                     AF.Sign, bias=mneg[:, co:co + 1], scale=1.0)
                outs.append(a)
            return outs

        # L2
        pt2 = tc.alloc_tile_pool(name="pt2", bufs=1)
        xh2, mneg2, Hq2 = conv_layer(2, [act1], "l2", 18, pt2)
        pa2 = tc.alloc_tile_pool(name="pa2", bufs=1, side="right")
        act2 = sign_to_padded(2, xh2, mneg2, Hq2, pa2)
        pt2.release()
        # L3
        pt3 = tc.alloc_tile_pool(name="pt3", bufs=1)
        xh3, mneg3, Hq3 = conv_layer(3, act2, "l3", 16, pt3,
                                     release_after_mm=pa2)
        pa3 = tc.alloc_tile_pool(name="pa3", bufs=1, side="right")
        act3 = sign_to_padded(3, xh3, mneg3, Hq3, pa3)
        pt3.release()
        # L4
        pt4 = tc.alloc_tile_pool(name="pt4", bufs=1)
        xh4, mneg4, Hq4 = conv_layer(4, act3, "l4", 16, pt4,
                                     release_after_mm=pa3)
        pa4 = tc.alloc_tile_pool(name="pa4", bufs=1, side="right")
        act4 = sign_to_padded(4, xh4, mneg4, Hq4, pa4)
        pt4.release()

        # prefetch fc weights early (fp8, 64KB/partition for wf1)
        pwf = tc.alloc_tile_pool(name="pwf", bufs=1)
        wf1_sb = pwf.tile([128, 64, 1024], FP8)
        nc.sync.dma_start(wf1_sb[:], wf1_p[:].rearrange("kt kp o -> kp kt o"))
        wf2_sb = pwf.tile([128, 8, 1024], BF16)
        nc.sync.dma_start(wf2_sb[:], wf2_p[:].rearrange("kt kp o -> kp kt o"))
        wf3_sb = pwf.tile([128, 8, 16], BF16)
        nc.sync.dma_start(wf3_sb[:], wf3_p[:].rearrange("kt kp o -> kp kt o"))
        gb3_sb = pwf.tile([16, 2], F32)
        nc.sync.dma_start(gb3_sb[:], gb3_p[:])

        # L5
        pt5 = tc.alloc_tile_pool(name="pt5", bufs=1)
        xh5, mneg5, Hq5 = conv_layer(5, act4, "l5", 14, pt5,
                                     release_after_mm=pa4)
        pa5 = tc.alloc_tile_pool(name="pa5", bufs=1, side="right")
        act5 = sign_to_padded(5, xh5, mneg5, Hq5, pa5)
        pt5.release()
        # L6 (output unpadded, fp8, then rearranged for fc1)
        pt6 = tc.alloc_tile_pool(name="pt6", bufs=1)
        xh6, mneg6, Hq6 = conv_layer(6, act5, "l6", 14, pt6,
                                     release_after_mm=pa5)
        pa6 = tc.alloc_tile_pool(name="pa6", bufs=1, side="right")
        act6 = []
        for co in range(4):
            a = pa6.tile([128, 16, B], FP8, name=f"act6_{co}")
            av = a[:].rearrange("p (y x) b -> p b y x", y=4, x=4)
            nc.scalar.activation(av, xh6[co][:], AF.Sign,
                                 bias=mneg6[:, co:co + 1], scale=1.0)
            act6.append(a)
        f1 = pa6.tile([128, 64, B], FP8)
        pt6.release()
        pfc = tc.alloc_tile_pool(name="pfct", bufs=1)

        # fc1 lhsT k-tiles: [128=(c8,pix16), B] fp8
        for kt in range(64):
            co = kt // 16
            p0 = (8 * kt) % 128
            for c in range(8):
                s = act6[co][p0 + c:p0 + c + 1, :, :]
                nc.sync.dma_start(f1[16 * c:16 * (c + 1), kt, :], s)

        # ---------------- fc helper ----------------
        def fc_stats_sign(xf, nch, key, out_bf16):
            """xf: [32, nch] f32 SBUF (batch on partitions).
            Transpose to [128, B] tiles, reduce over batch, AllReduce,
            Sign(x - m). Returns list of nch//128 bf16 [128, B] tiles."""
            J = nch // 128
            xT = pfc.tile([128, J, B], F32, name=f"xT_{key}")
            for j in range(J):
                pst = ps_tile(f"pst_{key}")
                nc.tensor.transpose(pst[0:128, 0:B],
                                    xf[:, 128 * j:128 * (j + 1)], ident32[:])
                nc.scalar.copy(xT[:, j, :], pst[0:128, 0:B])
            s = pfc.tile([128, J], F32, name=f"s_{key}")
            for j in range(J):
                nc.vector.reduce_sum(s[:, j:j + 1], xT[:, j, :], axis=AX.X)
                nc.sync.dma_start(arin[key][j, :], s[:, j:j + 1])
            allreduce(key)
            mn_ = pfc.tile([128, J], F32, name=f"mn_{key}")
            for j in range(J):
                nc.sync.dma_start(mn_[:, j:j + 1], arout[key][j, :])
            mneg_ = pfc.tile([128, J], F32, name=f"mneg_{key}")
            nc.scalar.mul(mneg_[:], mn_[:], -1.0 / NB_FULL)
            hs = []
            for j in range(J):
                h = pfc.tile([128, B], BF16, name=f"h_{key}_{j}",
                             tag=f"h_{key}")
                nc.scalar.activation(h[:], xT[:, j, :], AF.Sign,
                                     bias=mneg_[:, j:j + 1], scale=1.0)
                hs.append(h)
            return hs

        # fc1: out[b, o] = sum_k f1[k, b] * wf1[k, o]
        xf1 = pfc.tile([32, 1024], F32)
        for n in range(2):
            ps = ps_tile("f1ps")
            for kt in range(64):
                nc.tensor.matmul(ps[0:B, :], f1[:, kt, :],
                                 wf1_sb[:, kt, 512 * n:512 * (n + 1)],
                                 start=(kt == 0), stop=(kt == 63))
            nc.scalar.copy(xf1[:, 512 * n:512 * (n + 1)], ps[0:B, :])
        pa6.release()
        h2 = fc_stats_sign(xf1, 1024, "f1", True)

        # fc2
        xf2 = pfc.tile([32, 1024], F32)
        for n in range(2):
            ps = ps_tile("f2ps")
            for kt in range(8):
                nc.tensor.matmul(ps[0:B, :], h2[kt][:],
                                 wf2_sb[:, kt, 512 * n:512 * (n + 1)],
                                 start=(kt == 0), stop=(kt == 7))
            nc.scalar.copy(xf2[:, 512 * n:512 * (n + 1)], ps[0:B, :])
        h3 = fc_stats_sign(xf2, 1024, "f2", True)

        # fc3 (full BN, no sign)
        ps3 = ps_tile("f3ps")
        for kt in range(8):
            nc.tensor.matmul(ps3[0:B, 0:16], h3[kt][:], wf3_sb[:, kt, :],
                             start=(kt == 0), stop=(kt == 7))
        xf3 = pfc.tile([32, 16], F32)
        nc.scalar.copy(xf3[:], ps3[0:B, 0:16])
        pst3 = ps_tile("pst3")
        nc.tensor.transpose(pst3[0:16, 0:B], xf3[:], ident32[:])
        x3T = pfc.tile([16, B], F32)
        nc.scalar.copy(x3T[:], pst3[0:16, 0:B])
        s3 = pfc.tile([16, 2], F32)
        nc.vector.reduce_sum(s3[:, 0:1], x3T[:], axis=AX.X)
        x3sq = pfc.tile([16, B], F32)
        nc.scalar.square(x3sq[:], x3T[:])
        nc.vector.reduce_sum(s3[:, 1:2], x3sq[:], axis=AX.X)
        nc.sync.dma_start(arin["f3"][0, :], s3[:, 0:1])
        nc.sync.dma_start(arin["f3"][1, :], s3[:, 1:2])
        allreduce("f3")
        sg = pfc.tile([16, 2], F32)
        nc.sync.dma_start(sg[:, 0:1], arout["f3"][0, :])
        nc.sync.dma_start(sg[:, 1:2], arout["f3"][1, :])
        m3 = pfc.tile([16, 1], F32)
        nc.scalar.mul(m3[:], sg[:, 0:1], 1.0 / NB_FULL)
        m3n = pfc.tile([16, 1], F32)
        nc.scalar.mul(m3n[:], sg[:, 0:1], -1.0 / NB_FULL)
        ex2 = pfc.tile([16, 1], F32)
        nc.scalar.mul(ex2[:], sg[:, 1:2], 1.0 / NB_FULL)
        m3sq = pfc.tile([16, 1], F32)
        nc.vector.tensor_mul(m3sq[:], m3[:], m3[:])
        var3 = pfc.tile([16, 1], F32)
        nc.vector.tensor_sub(var3[:], ex2[:], m3sq[:])
        eps3 = pfc.tile([16, 1], F32)
        nc.vector.memset(eps3[:], 1e-5)
        sd3 = pfc.tile([16, 1], F32)
        nc.scalar.activation(sd3[:], var3[:], AF.Sqrt, bias=eps3[:], scale=1.0)
        rs3 = pfc.tile([16, 1], F32)
        nc.vector.reciprocal(rs3[:], sd3[:])
        # y = ((x - m) * g) * rs + beta   (matches ref order g*(x-m)*rs + b)
        t1 = pfc.tile([16, B], F32)
        nc.scalar.activation(t1[:], x3T[:], AF.Identity, bias=m3n[:], scale=1.0)
        t2 = pfc.tile([16, B], F32)
        nc.vector.tensor_scalar(t2[:], t1[:], gb3_sb[:, 0:1], rs3[:],
                                op0=mybir.AluOpType.mult,
                                op1=mybir.AluOpType.mult)
        y3 = pfc.tile([16, B], F32)
        nc.vector.tensor_scalar(y3[:], t2[:], gb3_sb[:, 1:2], None,
                                op0=mybir.AluOpType.add)
        psy = ps_tile("psy")
        nc.tensor.transpose(psy[0:B, 0:16], y3[:], ident32[0:16, 0:16])
        yout = pfc.tile([32, 16], F32)
        nc.scalar.copy(yout[:], psy[0:B, 0:16])
        nc.sync.dma_start(y_p[:], yout[:, 0:10])

        pfc.release()
        pwf.release()
        poolA1.release()
        pconst.release()
        psum.release()
        dram.release()

    nc.finalize()
    return nc


# ======================= host side =======================

def _prep_inputs(x, conv_params, fc_params):
    """Build per-core in_maps. Returns list of dicts."""
    import ml_dtypes  # noqa
    f8 = mybir.dt.np(FP8)
    bf = mybir.dt.np(BF16)

    x = np.asarray(x, np.float32)
    # hi/lo split on 2^-14 grid (exact in f32; xh+xl == x exactly)
    x64 = x.astype(np.float64)
    xh = (np.round(x64 * (1 << 14)) / (1 << 14)).astype(np.float32)
    xl = (x64 - xh.astype(np.float64)).astype(np.float32)
    assert np.all(xh.astype(np.float64) + xl.astype(np.float64) == x64)

    def pad_cbhw(a):  # [B?,3,32,32] -> [3, B?, 34, 34]
        B_ = a.shape[0]
        out = np.zeros((3, B_, 34, 34), np.float32)
        out[:, :, 1:33, 1:33] = a.transpose(1, 0, 2, 3)
        return out

    w1 = np.sign(np.asarray(conv_params[0][0], np.float32))
    wc1 = w1.reshape(128, 27).T.copy()  # [27(c,ky,kx), 128]

    w_l = {}
    for l in range(2, 7):
        cfg = CONV_CFG[l]
        w = np.sign(np.asarray(conv_params[l - 1][0], np.float32))
        CO, CI = cfg["co"], cfg["ci"]
        # [CO*128, CI*128, 3, 3] -> [CI, 9, CO, 128kc, 128ko]
        wr = w.reshape(CO, 128, CI, 128, 3, 3)
        w_l[l] = np.ascontiguousarray(
            wr.transpose(2, 4, 5, 0, 3, 1)  # CI,ky,kx,CO,kc,ko
            .reshape(CI, 9, CO, 128, 128)).astype(f8)

    wf1 = np.sign(np.asarray(fc_params[0][0], np.float32))  # [1024, 8192]
    wf1 = np.ascontiguousarray(wf1.T.reshape(64, 128, 1024)).astype(f8)
    wf2 = np.sign(np.asarray(fc_params[1][0], np.float32))
    wf2 = np.ascontiguousarray(wf2.T.reshape(8, 128, 1024)).astype(bf)
    wf3s = np.sign(np.asarray(fc_params[2][0], np.float32))  # [10, 1024]
    wf3 = np.zeros((1024, 16), np.float32)
    wf3[:, 0:10] = wf3s.T
    wf3 = np.ascontiguousarray(wf3.reshape(8, 128, 16)).astype(bf)
    gb3 = np.zeros((16, 2), np.float32)
    gb3[:, 0] = 1.0
    gb3[0:10, 0] = np.asarray(fc_params[2][2], np.float32)
    gb3[0:10, 1] = np.asarray(fc_params[2][3], np.float32)

    in_maps = []
    for c in range(N_CORES):
        sl = slice(c * B, (c + 1) * B)
        in_maps.append({
            "xh": pad_cbhw(xh[sl]), "xl": pad_cbhw(xl[sl]),
            "wc1": wc1,
            **{f"w{l}": w_l[l] for l in range(2, 7)},
            "wf1": wf1, "wf2": wf2, "wf3": wf3, "gb3": gb3,
        })
    return in_maps


def _special_case_ok(conv_params, fc_params):
    for (w, b, g, be) in list(conv_params) + list(fc_params)[:2]:
        if not (np.all(np.asarray(b) == 0) and np.all(np.asarray(g) == 1)
                and np.all(np.asarray(be) == 0)):
            return False
    w, b, g, be = fc_params[2]
    if not np.all(np.asarray(b) == 0):
        return False
    if not np.all(np.asarray(g) > 0):
        return False
    return True


def _np_reference(x, conv_params, fc_params):
    """Slow exact fallback (float64 numpy) for non-special-case params."""
    EPS = 1e-5
    DT = np.float64

    def conv3x3(h, w):
        Bn, C, H, W = h.shape
        O = w.shape[0]
        hp = np.zeros((Bn, C, H + 2, W + 2), DT)
        hp[:, :, 1:-1, 1:-1] = h
        cols = np.empty((Bn, H, W, C, 9), DT)
        for ky in range(3):
            for kx in range(3):
                cols[..., ky * 3 + kx] = hp[:, :, ky:ky + H, kx:kx + W] \
                    .transpose(0, 2, 3, 1)
        y = cols.reshape(Bn * H * W, C * 9) @ \
            w.transpose(1, 2, 3, 0).reshape(C * 9, O)
        return y.reshape(Bn, H, W, O).transpose(0, 3, 1, 2)

    h = np.asarray(x, DT)
    for li, ((w, b, g, be), dp) in enumerate(zip(
            conv_params, (False, True, False, True, False, True))):
        w = np.asarray(w, DT)
        h = conv3x3(h, np.sign(w)) + np.asarray(b, DT)[None, :, None, None]
        m = h.mean(axis=(0, 2, 3), keepdims=True)
        v = ((h - m) ** 2).mean(axis=(0, 2, 3), keepdims=True)
        h = np.asarray(g, DT)[None, :, None, None] * (h - m) / \
            np.sqrt(v + EPS) + np.asarray(be, DT)[None, :, None, None]
        if dp:
            Bn, C, H, W = h.shape
            h = h.reshape(Bn, C, H // 2, 2, W // 2, 2).max(axis=(3, 5))
        h = np.sign(h)
    h = h.reshape(h.shape[0], -1)
    for li, (w, b, g, be) in enumerate(fc_params):
        h = h @ np.sign(np.asarray(w, DT)).T + np.asarray(b, DT)[None, :]
        m = h.mean(axis=0, keepdims=True)
        v = ((h - m) ** 2).mean(axis=0, keepdims=True)
        h = np.asarray(g, DT)[None, :] * (h - m) / np.sqrt(v + EPS) + \
            np.asarray(be, DT)[None, :]
        if li < 2:
            h = np.sign(h)
    return h.astype(np.float32)


def get_nc():
    if "nc" not in _RUNNER:
        _RUNNER["nc"] = build()
    return _RUNNER["nc"]


def kernel(x, conv_params, fc_params):
    x = np.asarray(x, np.float32)
    if not _special_case_ok(conv_params, fc_params):
        return _np_reference(x, conv_params, fc_params)
    nc = get_nc()
    in_maps = _prep_inputs(x, conv_params, fc_params)
    res = run_bass_kernel_spmd(nc, in_maps, list(range(N_CORES)))
    return np.concatenate([res.results[c]["y"] for c in range(N_CORES)],
                          axis=0)
